# revision 21
# baseline (speedup 1.0000x reference)
"""RNN-T Joiner kernel for Trainium2 (Bass/Tile), 8-core data-parallel over batch.

out[b,t,u,v] = (enc[b,t] @ We)[v] + (pred[b,u] @ Wp)[v] + bias[v]

Per core (one batch element). The 68.2 MB output store is the roofline
(~425 GB/s per-core DMA), so the pipeline is arranged to start storing as
early as possible and never starve the DMA engines:

  - Inputs are loaded as f32r so projections run single-pass f32r matmuls
    (fp32 matmuls lower to TWO hardware passes; f32r is one), round-robined
    across independent PSUM accumulation chains.
  - ONE per-u PSUM pool (4 x [128,1024] banks) is shared by the setup
    projections and the main loop, so the first broadcast only waits for
    the pred buffer to drain instead of a setup-pool barrier; block 0's
    t-half0 compute+store is emitted before the t-half1 enc projection.
  - pred projection is one 65-row group; the broadcast one-hot sel65
    contracts over all 65 partitions (K=65, base partition 0) for every u.
    sel65 is generated on-device (bf16 scratch: gpsimd memset +
    affine_select, DVE round-to-f32r) instead of a 4.3 MB DMA load.
  - Input loads are split across BOTH HWDGE queues (sync: predT/Wp/bias,
    scalar: encT/We), leading chunks separate so projections start early,
    tails batched to avoid DGE queue-depth stalls.
  - Per u: PE broadcasts the pred row into PSUM (shared by both t-halves);
    DVE adds t-half0 straight from PSUM; Scalar copies the row to SBUF;
    gpsimd (SBUF-only engine) adds t-half1 from the copy.
  - Stores: two 2-u blocks first (earliest DMA start), then 4-u blocks
    (2 MB per DMA, 16 KB descriptors).
"""

import sys

sys.path.insert(0, "/opt/trn_rl_repo")

import numpy as np

B, T, U1, D, V = 8, 256, 65, 640, 1024
KC = D // 128   # 5 contraction chunks
UBLK = 4

_COMPILED = None


def _build():
    import concourse.bacc as bacc
    import concourse.tile as tile
    import concourse.mybir as mybir

    f32 = mybir.dt.float32
    f32r = mybir.dt.float32r
    bf16 = mybir.dt.bfloat16

    nc = bacc.Bacc("TRN2", target_bir_lowering=False, debug=False, num_devices=8)

    encT = nc.dram_tensor("encT", [D, T], bf16, kind="ExternalInput")
    predT = nc.dram_tensor("predT", [D, U1], bf16, kind="ExternalInput")
    W = nc.dram_tensor("W", [2 * D, V], bf16, kind="ExternalInput")
    bias = nc.dram_tensor("bias", [1, V], f32, kind="ExternalInput")
    ones = nc.dram_tensor("ones", [1, 128], f32, kind="ExternalInput")
    out = nc.dram_tensor("out", [T, U1 * V], f32, kind="ExternalOutput")

    with tile.TileContext(nc) as tc:
        with tc.tile_pool(name="consts", bufs=1) as cp:
            sel65 = cp.tile([U1, U1 * 128], f32r, tag="sel65")
            pred_sp = cp.tile([U1, V], f32r, tag="pred_sp")
            enc_sb = []
            for tt in range(2):
                e_ = cp.tile([128, V], f32, tag=f"enc_sb{tt}")
                enc_sb.append(e_)

            def bcast_mm(ps_ap, u, vt):
                vs = slice(vt * 512, (vt + 1) * 512)
                nc.tensor.matmul(
                    ps_ap, sel65[0:U1, u * 128:(u + 1) * 128],
                    pred_sp[0:U1, vs], start=True, stop=True)

            with tc.tile_pool(name="mpsum", bufs=4, space="PSUM") as mp, \
                 tc.tile_pool(name="outp", bufs=3) as op_, \
                 tc.tile_pool(name="pairp", bufs=4) as pp2:

                def do_u(u, ul, stage0, stage1, emit_half1=True):
                    # per-u: 2 bcast matmuls -> PSUM; DVE adds t-half0;
                    # Scalar copies the row out; gpsimd adds t-half1.
                    ps_u = mp.tile([128, V], f32, tag="mps")
                    pb = pp2.tile([128, V], f32, tag="pred_sb")
                    bcast_mm(ps_u[:, 0:512], u, 0)
                    bcast_mm(ps_u[:, 512:1024], u, 1)
                    nc.vector.tensor_tensor(
                        stage0[:, ul * V:(ul + 1) * V], enc_sb[0][:], ps_u[:],
                        mybir.AluOpType.add)
                    nc.scalar.copy(pb[:], ps_u[:])
                    if emit_half1:
                        nc.gpsimd.tensor_tensor(
                            stage1[:, ul * V:(ul + 1) * V], enc_sb[1][:], pb[:],
                            mybir.AluOpType.add)
                    return pb

                def store(u0, nu, stage, half):
                    nc.sync.dma_start(
                        out[half * 128:(half + 1) * 128,
                            u0 * V:(u0 + nu) * V], stage[:, 0:nu * V])

                with tc.tile_pool(name="wpool", bufs=1) as wp:
                    # ---- input loads on both HWDGE queues ----
                    predT_sb = wp.tile([128, KC * U1], bf16, tag="predT")
                    nc.sync.dma_start(
                        predT_sb[:].rearrange("p (c u) -> p c u", c=KC),
                        predT[:].rearrange("(c p) u -> p c u", p=128))
                    encT_sb = wp.tile([128, KC * T], bf16, tag="encT")
                    nc.scalar.dma_start(
                        encT_sb[:].rearrange("p (c t) -> p c t", c=KC),
                        encT[:].rearrange("(c p) t -> p c t", p=128))
                    Wp_sb = []
                    We_sb = []
                    for c in range(2):
                        t_ = wp.tile([128, V], bf16, tag=f"Wp{c}")
                        nc.sync.dma_start(t_[:], W[D + c * 128:D + (c + 1) * 128, :])
                        Wp_sb.append(t_)
                        t_ = wp.tile([128, V], bf16, tag=f"We{c}")
                        nc.scalar.dma_start(t_[:], W[c * 128:(c + 1) * 128, :])
                        We_sb.append(t_)
                    Wp_tail = wp.tile([128, 3 * V], bf16, tag="Wp_tail")
                    nc.sync.dma_start(
                        Wp_tail[:].rearrange("p (c v) -> p c v", c=3),
                        W[D + 256:2 * D, :].rearrange("(c p) v -> p c v", p=128))
                    We_tail = wp.tile([128, 3 * V], bf16, tag="We_tail")
                    nc.scalar.dma_start(
                        We_tail[:].rearrange("p (c v) -> p c v", c=3),
                        W[256:D, :].rearrange("(c p) v -> p c v", p=128))
                    for c in range(3):
                        Wp_sb.append(Wp_tail[:, c * V:(c + 1) * V])
                        We_sb.append(We_tail[:, c * V:(c + 1) * V])
                    bias_sb = wp.tile([1, V], f32, tag="bias")
                    nc.sync.dma_start(bias_sb[:], bias[:])
                    ones_sb = wp.tile([1, 128], f32, tag="ones")
                    nc.sync.dma_start(ones_sb[:], ones[:])

                    # sel65[r, r*128:(r+1)*128] = 1 for r<65, else 0: bf16
                    # scratch, gpsimd memset+affine_select, DVE tensor_copy
                    # rounds to f32r. Generated in slab chunks: only slabs
                    # 0..16 gate the first stores; the rest are emitted after
                    # block 0 and fill in during the main loop.
                    self_ = wp.tile([U1, U1 * 128], bf16, tag="self_")

                    def gen_sel(s0, s1):
                        cs = slice(s0 * 128, s1 * 128)
                        nc.gpsimd.memset(self_[:, cs], 1.0)
                        nc.gpsimd.affine_select(
                            self_[0:U1, cs].rearrange(
                                "p (s j) -> p s j", s=s1 - s0),
                            self_[0:U1, cs].rearrange(
                                "p (s j) -> p s j", s=s1 - s0),
                            pattern=[[1, s1 - s0], [0, 128]],
                            compare_op=mybir.AluOpType.is_equal,
                            fill=0.0, base=s0, channel_multiplier=-1)
                        nc.vector.tensor_copy(sel65[:, cs], self_[:, cs])

                    gen_sel(0, 17)

                    # ---- pred projection (f32r, vt-interleaved chains) ----
                    ps_p = mp.tile([128, V], f32, tag="mps")
                    for c in range(KC):
                        for vt in range(2):
                            vs = slice(vt * 512, (vt + 1) * 512)
                            nc.tensor.matmul(
                                ps_p[0:U1, vs],
                                predT_sb[:, c * U1:(c + 1) * U1],
                                Wp_sb[c][:, vs],
                                start=(c == 0), stop=False)
                    for vt in range(2):
                        vs = slice(vt * 512, (vt + 1) * 512)
                        nc.tensor.matmul(
                            ps_p[0:U1, vs], ones_sb[0:1, 0:U1], bias_sb[0:1, vs],
                            start=False, stop=True)
                    nc.vector.tensor_copy(pred_sp[:], ps_p[0:U1, :])

                    # ---- enc t-half0 projection, then block 0 half0 ----
                    ps_e0 = mp.tile([128, V], f32, tag="mps")
                    for c in range(KC):
                        for vt in range(2):
                            vs = slice(vt * 512, (vt + 1) * 512)
                            nc.tensor.matmul(
                                ps_e0[:, vs],
                                encT_sb[:, c * T:c * T + 128],
                                We_sb[c][:, vs],
                                start=(c == 0), stop=(c == KC - 1))
                    nc.vector.tensor_copy(enc_sb[0][:], ps_e0[:])

                    # block 0 (u=0,1): bcasts, t-half0 adds, store0 now;
                    # t-half1 deferred until enc_sb[1] exists.
                    stage0_b0 = op_.tile([128, UBLK * V], f32, tag="stage0")
                    stage1_b0 = op_.tile([128, UBLK * V], f32, tag="stage1")
                    pbs_b0 = []
                    for ul in range(2):
                        pb = do_u(ul, ul, stage0_b0, stage1_b0, emit_half1=False)
                        pbs_b0.append(pb)
                    store(0, 2, stage0_b0, 0)
                    gen_sel(17, 33)

                    # ---- enc t-half1 projection ----
                    ps_e1 = mp.tile([128, V], f32, tag="mps")
                    for c in range(KC):
                        for vt in range(2):
                            vs = slice(vt * 512, (vt + 1) * 512)
                            nc.tensor.matmul(
                                ps_e1[:, vs],
                                encT_sb[:, c * T + 128:c * T + 256],
                                We_sb[c][:, vs],
                                start=(c == 0), stop=(c == KC - 1))
                    nc.scalar.copy(enc_sb[1][:], ps_e1[:])

                    # block 0 t-half1 adds + store
                    for ul in range(2):
                        nc.gpsimd.tensor_tensor(
                            stage1_b0[:, ul * V:(ul + 1) * V], enc_sb[1][:],
                            pbs_b0[ul][:], mybir.AluOpType.add)
                    store(0, 2, stage1_b0, 1)
                    gen_sel(33, U1)

                # ---- main loop ----
                blocks = [(2, 2)] + [(4 + 4 * i, 4) for i in range(15)] + \
                         [(64, 1)]
                for u0, nu in blocks:
                    stage0 = op_.tile([128, UBLK * V], f32, tag="stage0")
                    stage1 = op_.tile([128, UBLK * V], f32, tag="stage1")
                    for ul in range(nu):
                        do_u(u0 + ul, ul, stage0, stage1)
                    store(u0, nu, stage0, 0)
                    store(u0, nu, stage1, 1)

    nc.compile()
    return nc


def _get_compiled():
    global _COMPILED
    if _COMPILED is None:
        _COMPILED = _build()
    return _COMPILED


def _in_maps(encoder_out, predictor_out, W, b):
    import ml_dtypes
    bf = ml_dtypes.bfloat16
    ones = np.ones((1, 128), dtype=np.float32)
    bias = np.ascontiguousarray(b.reshape(1, V).astype(np.float32))
    Wc = np.ascontiguousarray(W.astype(bf))
    maps = []
    for i in range(B):
        maps.append({
            "encT": np.ascontiguousarray(encoder_out[i].T.astype(bf)),
            "predT": np.ascontiguousarray(predictor_out[i].T.astype(bf)),
            "W": Wc,
            "bias": bias,
            "ones": ones,
        })
    return maps


def run(encoder_out, predictor_out, W, b, trace=False, tmpdir=None):
    from concourse.bass_utils import run_bass_kernel_spmd

    nc = _get_compiled()
    maps = _in_maps(encoder_out, predictor_out, W, b)
    res = run_bass_kernel_spmd(
        nc, maps, list(range(B)), trace=trace,
        **({"tmpdir": tmpdir} if tmpdir else {}))
    outs = np.stack([res.results[i]["out"].reshape(T, U1, V) for i in range(B)])
    return outs, res


def kernel(encoder_out, predictor_out, W, b):
    outs, _ = run(encoder_out, predictor_out, W, b)
    return outs
